# revision 53
# baseline (speedup 1.0000x reference)
"""LRU (Linear Recurrent Unit) single-step forward on 8 Trainium2 NeuronCores.

Math: with seq-len 1 the whole LRU step collapses algebraically to one GEMM:
    y[b,:] = W @ u[b] + bias
(W, bias folded on host in float64; see _fold_params).  The batch GEMM runs
on the 8 NeuronCores, data-parallel over the batch: each core computes
y_shard^T = W @ u_shard^T (+bias), a 2048x1024x1024 GEMM.

Mixed-precision split (norm rel err 1.6e-2 vs the 2e-2 gate, verified
offline against the exact quantization):
  - Contraction range 0:256 runs in fp8e4m3 with MatmulPerfMode.DoubleRow
    (2 MACs/cell/cycle): operands scaled by 16 (u) and 16 (W).
  - Contraction range 256:1024 runs in fp16: u unscaled, W scaled by 256.
  - All partial products therefore carry exactly 2^8 (power-of-2 scales are
    exact in fp8/fp16), so both parts accumulate in ONE fp32 PSUM group;
    the PSUM->SBUF drain rescales by 2^-8 and adds the bias.
This cuts the PE stream from 256 to 32 DoubleRow + 192 fp16 matmuls
(~49us vs the 55.3us all-fp16 floor).

Pipeline (per core):
  - Combined (u-block, W-block) "pair" loads: one DMA per contraction
    block -> exactly one DMAHW-lane semaphore gates each K-outer matmul
    group (avoids Tile's wait-merging: 8 round-robin lanes, 1 wait slot
    per matmul).  The fp8 head pair is split so the first matmul group
    gates on 256 KiB.
  - Loads alternate across both HWDGE queues (sync ring first: measured
    first-byte ~0.65us vs ~1.5us for scalar's first use); u tiles for
    batch-tiles 1-3 ride per-deadline-chosen queues.
  - PE warm-up junk matmuls gated only by two tiny DVE memsets cover the
    DMA-latency launch window and release the HAM clock gate (~3.4us
    continuous-activity window) as the first data lands.
  - PSUM->SBUF drains on DVE (x2^-8, +bias, fp16 cast via tensor_scalar);
    stores per jb-pair alternate across both queues.  The very last
    matmul group is split into two half-width PSUM groups (ACT drains the
    first half in parallel; act-table preloaded by an early ACT drain),
    leaving ~380ns half-drain + one 32 KiB store after the final matmul.
y returns as fp16 and is upcast on host.

Remaining fixed overhead outside kernel control: the NEFF wrapper's
epilogue (~8.5us: one-at-a-time sweep zeroing all 256 semaphores) and the
~4-6us DMA-latency launch window (hidden behind warm-up).
"""

import numpy as np

BATCH, DIN, DSTATE, DOUT = 16384, 1024, 2048, 1024
N_CORES = 8
B_SHARD = BATCH // N_CORES  # 2048 rows per core
P = 128                     # SBUF partitions
NB = 512                    # batch tile (moving free dim, max 512 per PSUM bank)
I_BLOCKS = DIN // P         # 8 contraction blocks (block 0:2 fused in fp8)
J_BLOCKS = DOUT // P        # 8 output-row blocks
B_TILES = B_SHARD // NB     # 4 batch tiles per core
N_WARM = 17                 # PE warm-up matmuls (HAM clock-gate release)
F16_IB0 = 2                 # first fp16 contraction block index
N_F16 = I_BLOCKS - F16_IB0  # 6 fp16 blocks
SU8, SW8, SW16 = 16.0, 16.0, 256.0   # su8*sw8 == 1*sw16 == 2^8
SCALE_INV = 1.0 / 256.0

_CACHE = {}


def _build_nc():
    import concourse.mybir as mybir
    import concourse.tile as tile
    from concourse import bacc
    from concourse._compat import get_trn_type

    nc = bacc.Bacc(get_trn_type() or "TRN2", target_bir_lowering=False)
    f32 = mybir.dt.float32
    f16 = mybir.dt.float16
    f8 = mybir.dt.float8e4
    dr = mybir.MatmulPerfMode.DoubleRow

    # fp8 head: [p][k2][0:NB u8 | NB:NB+DOUT W8] for contraction 0:256
    head8 = nc.declare_dram_parameter("head8", [P, 2, NB + DOUT], f8,
                                      isOutput=False)
    # fp16 pairs for contraction blocks 2..7: [p][0:NB u | NB:NB+DOUT W]
    head = nc.declare_dram_parameter("head", [N_F16, P, NB + DOUT], f16,
                                     isOutput=False)
    # u batch-tiles 1-3: fp16 part (blocks 2..7) and fp8 part (0:256)
    ubr = nc.declare_dram_parameter("ubr", [B_TILES - 1, P, N_F16 * NB],
                                    f16, isOutput=False)
    ubr8 = nc.declare_dram_parameter("ubr8", [B_TILES - 1, P, 2, NB], f8,
                                     isOutput=False)
    bias = nc.declare_dram_parameter("bias", [P, J_BLOCKS], f32,
                                     isOutput=False)
    yb = nc.declare_dram_parameter("yb", [B_TILES, P, J_BLOCKS * NB], f16,
                                   isOutput=True)

    with tile.TileContext(nc) as tc:
        with (
            tc.tile_pool(name="consts", bufs=1) as consts,
            tc.tile_pool(name="upool", bufs=1) as upool,
            tc.tile_pool(name="ypool", bufs=1) as ypool,
            tc.tile_pool(name="psum", bufs=8, space="PSUM") as psum,
        ):
            warm_w = consts.tile([P, P], f16, tag="warm_w")
            warm_u = consts.tile([P, 256], f16, tag="warm_u")
            nc.vector.memset(warm_w[:], 0.0)
            nc.vector.memset(warm_u[:], 0.0)
            warm_p = psum.tile([P, NB], f32, tag="pt", name="warm_p")
            for _ in range(N_WARM):
                nc.tensor.matmul(warm_p[:, 0:256], warm_w[:], warm_u[:],
                                 start=True, stop=True)

            # fp8 head split: the first matmul group (jb0-3) gates on
            # u8 + W8 columns 0:512 (256 KiB)
            pair8_t = consts.tile([P, 2, NB + DOUT], f8, tag="pair8",
                                  name="pair8")
            h = NB + DOUT // 2
            nc.sync.dma_start(out=pair8_t[:, :, 0:h], in_=head8[:, :, 0:h])
            pair_tiles = []
            for hib in range(N_F16):
                pt_ = consts.tile([P, NB + DOUT], f16, tag=f"pair{hib}",
                                  name=f"pair{hib}")
                pair_tiles.append(pt_)
            nc.scalar.dma_start(out=pair_tiles[0][:], in_=head[0])
            nc.sync.dma_start(out=pair8_t[:, :, h:NB + DOUT],
                              in_=head8[:, :, h:NB + DOUT])
            nc.scalar.dma_start(out=pair_tiles[1][:], in_=head[1])
            nc.sync.dma_start(out=pair_tiles[2][:], in_=head[2])
            nc.scalar.dma_start(out=pair_tiles[3][:], in_=head[3])
            nc.sync.dma_start(out=pair_tiles[4][:], in_=head[4])
            nc.scalar.dma_start(out=pair_tiles[5][:], in_=head[5])
            bias_t = consts.tile([P, J_BLOCKS], f32, tag="bias")
            nc.sync.dma_start(out=bias_t[:], in_=bias[:])
            ur8_tiles = []
            ur_tiles = []
            for r in range(B_TILES - 1):
                u8t = upool.tile([P, 2, NB], f8, tag=f"ur8_{r}",
                                 name=f"ur8_{r}")
                ur8_tiles.append(u8t)
                urt = upool.tile([P, N_F16 * NB], f16, tag=f"ur{r}",
                                 name=f"ur{r}")
                ur_tiles.append(urt)
            # bt1 (tightest deadline) on sync; spread the rest
            nc.scalar.dma_start(out=ur8_tiles[0][:], in_=ubr8[0])
            nc.sync.dma_start(out=ur_tiles[0][:], in_=ubr[0])
            nc.scalar.dma_start(out=ur_tiles[1][:], in_=ubr[1])
            nc.sync.dma_start(out=ur8_tiles[1][:], in_=ubr8[1])
            nc.scalar.dma_start(out=ur8_tiles[2][:], in_=ubr8[2])
            nc.sync.dma_start(out=ur_tiles[2][:], in_=ubr[2])

            def w8_block(jb):
                return pair8_t[:, :, NB + jb * P:NB + (jb + 1) * P]

            def w_block(ib, jb):
                return pair_tiles[ib - F16_IB0][:, NB + jb * P:
                                                NB + (jb + 1) * P]

            def u0_block(ib):
                return pair_tiles[ib - F16_IB0][:, 0:NB]

            y_tiles = [ypool.tile([P, J_BLOCKS * NB], f16, tag=f"y{bt}",
                                  name=f"y{bt}")
                       for bt in range(B_TILES)]

            act_id = mybir.ActivationFunctionType.Identity
            op_mul = mybir.AluOpType.mult
            op_add = mybir.AluOpType.add

            def drain_store(bt, jb, pt):
                """PSUM -> SBUF: x2^-8, +bias, fp16 cast; then store.

                Drains alternate DVE (even jb) / ACT (odd jb) so the 8-bank
                recycle keeps pace with the next batch-tile's back-to-back
                DoubleRow layer; stores ride the sync queue (idle mid-stream)
                so ACT's drain chain is never delayed by descriptor gen."""
                yt = y_tiles[bt]
                if jb % 2 == 1:
                    nc.scalar.activation(yt[:, jb * NB:(jb + 1) * NB], pt[:],
                                         act_id, bias=bias_t[:, jb:jb + 1],
                                         scale=SCALE_INV)
                else:
                    nc.vector.tensor_scalar(yt[:, jb * NB:(jb + 1) * NB],
                                            pt[:], SCALE_INV,
                                            bias_t[:, jb:jb + 1],
                                            op_mul, op_add)
                if bt == B_TILES - 1:
                    q = nc.sync if jb % 2 == 0 else nc.scalar
                    q.dma_start(
                        out=yb[bt, :, jb * NB:(jb + 1) * NB],
                        in_=yt[:, jb * NB:(jb + 1) * NB])
                elif jb % 2 == 1:
                    nc.sync.dma_start(
                        out=yb[bt, :, (jb - 1) * NB:(jb + 1) * NB],
                        in_=yt[:, (jb - 1) * NB:(jb + 1) * NB])

            # Batch tile 0 runs K-outer: the fp8 DoubleRow layer opens all 8
            # PSUM groups, then the 6 fp16 layers accumulate.
            pts = [psum.tile([P, NB], f32, tag="pt", name=f"pt_0_{jb}")
                   for jb in range(J_BLOCKS)]
            for jb in range(J_BLOCKS):
                nc.tensor.matmul(pts[jb][:], w8_block(jb),
                                 pair8_t[:, :, 0:NB],
                                 start=True, stop=False, perf_mode=dr)
            for ib in range(F16_IB0, I_BLOCKS):
                for jb in range(J_BLOCKS):
                    nc.tensor.matmul(
                        pts[jb][:],
                        w_block(ib, jb),
                        u0_block(ib),
                        start=False,
                        stop=(ib == I_BLOCKS - 1),
                    )
            for jb in range(J_BLOCKS):
                drain_store(0, jb, pts[jb])

            # Batch tiles 1-3 run in 4-group blocks: the DoubleRow matmuls of
            # a block issue back-to-back (isolated DR matmuls pace at ~407ns;
            # consecutive ones pipeline at ~250ns), then the fp16 layers,
            # then the block's drains (which overlap the next block).
            for bt in range(1, B_TILES):
                ur8 = ur8_tiles[bt - 1]
                ur = ur_tiles[bt - 1]
                last_bt = bt == B_TILES - 1
                blocks = ([tuple(range(7))] if last_bt
                          else [tuple(range(8))])
                for blk in blocks:
                    bpts = {}
                    for jb in blk:
                        pt = psum.tile([P, NB], f32, tag="pt",
                                       name=f"pt_{bt}_{jb}")
                        nc.tensor.matmul(pt[:], w8_block(jb),
                                         ur8[:, :, 0:NB],
                                         start=True, stop=False,
                                         perf_mode=dr)
                        bpts[jb] = pt
                    for ib in range(F16_IB0, I_BLOCKS):
                        for jb in blk:
                            nc.tensor.matmul(
                                bpts[jb][:],
                                w_block(ib, jb),
                                ur[:, (ib - F16_IB0) * NB:
                                   (ib - F16_IB0 + 1) * NB],
                                start=False,
                                stop=(ib == I_BLOCKS - 1),
                            )
                    for jb in blk:
                        drain_store(bt, jb, bpts[jb])
                if last_bt:
                    # final jb7: two half-width PSUM groups (DRs back-to-back)
                    # for a short kernel tail after the last matmul
                    jb = J_BLOCKS - 1
                    hh = NB // 2
                    yt = y_tiles[bt]
                    pths = []
                    for half in range(2):
                        pth = psum.tile([P, NB], f32, tag="pt",
                                        name=f"pt_{bt}_{jb}_{half}")
                        nc.tensor.matmul(
                            pth[:, 0:hh], w8_block(jb),
                            ur8[:, :, half * hh:half * hh + hh],
                            start=True, stop=False, perf_mode=dr)
                        pths.append(pth)
                    for ib in range(F16_IB0, I_BLOCKS):
                        for half in range(2):
                            o = (ib - F16_IB0) * NB + half * hh
                            nc.tensor.matmul(
                                pths[half][:, 0:hh],
                                w_block(ib, jb),
                                ur[:, o:o + hh],
                                start=False,
                                stop=(ib == I_BLOCKS - 1),
                            )
                    for half in range(2):
                        c0 = jb * NB + half * hh
                        if half == 0:
                            nc.scalar.activation(
                                yt[:, c0:c0 + hh], pths[half][:, 0:hh],
                                act_id, bias=bias_t[:, jb:jb + 1],
                                scale=SCALE_INV)
                            nc.scalar.dma_start(
                                out=yb[bt, :, c0:c0 + hh],
                                in_=yt[:, c0:c0 + hh])
                        else:
                            nc.vector.tensor_scalar(
                                yt[:, c0:c0 + hh], pths[half][:, 0:hh],
                                SCALE_INV, bias_t[:, jb:jb + 1],
                                op_mul, op_add)
                            nc.sync.dma_start(
                                out=yb[bt, :, c0:c0 + hh],
                                in_=yt[:, c0:c0 + hh])
    nc.compile()
    return nc


def _fold_params(x_re, x_im, nu_log, theta_log, gamma_log, B_re, B_im, C_re, C_im, D):
    """Fold the LRU parameters into (W^T [DIN, DOUT], bias [DOUT]) in float64."""
    nu = np.asarray(nu_log, np.float64)
    th = np.exp(np.asarray(theta_log, np.float64))
    lam_mod = np.exp(-np.exp(nu))
    lam_re = lam_mod * np.cos(th)
    lam_im = lam_mod * np.sin(th)
    g = np.exp(np.asarray(gamma_log, np.float64))
    C_re64 = np.asarray(C_re, np.float64)
    C_im64 = np.asarray(C_im, np.float64)
    W = (2.0 * ((C_re64 * g) @ np.asarray(B_re, np.float64))
         - 2.0 * ((C_im64 * g) @ np.asarray(B_im, np.float64))
         + np.asarray(D, np.float64))  # [DOUT, DIN]
    xr = np.asarray(x_re, np.float64)
    xi = np.asarray(x_im, np.float64)
    lx_re = lam_re * xr - lam_im * xi
    lx_im = lam_re * xi + lam_im * xr
    bias = 2.0 * (C_re64 @ lx_re - C_im64 @ lx_im)  # [DOUT]
    return W.T.astype(np.float32).copy(), bias.astype(np.float32)


def kernel(u_in, x_re, x_im, nu_log, theta_log, gamma_log, B_re, B_im,
           C_re, C_im, D, _trace=False):
    from concourse.bass_utils import run_bass_kernel_spmd
    import concourse.mybir as mybir

    f8np = mybir.dt.np(mybir.dt.float8e4)

    wt_host, bias_host = _fold_params(
        x_re, x_im, nu_log, theta_log, gamma_log, B_re, B_im, C_re, C_im, D)
    bias2 = np.ascontiguousarray(bias_host.reshape(J_BLOCKS, P).T)  # [128, 8]

    K8 = F16_IB0 * P   # 256 contraction indices in fp8
    # W8c[p, k2, j] = W^T[k2*P+p, j] * SW8   (fp8)
    W8c = ((wt_host[0:K8] * SW8).reshape(2, P, DOUT)
           .transpose(1, 0, 2).astype(f8np))
    # wt16[hib, p, j] = W^T[K8+hib*P+p, j] * SW16  (fp16)
    wt16 = ((wt_host[K8:] * SW16).astype(np.float16)
            .reshape(N_F16, P, DOUT))

    u32 = np.asarray(u_in, np.float32).reshape(BATCH, DIN)
    core_ids = list(range(N_CORES))
    in_maps = []
    for c in core_ids:
        shard = u32[c * B_SHARD:(c + 1) * B_SHARD]          # [2048, 1024]
        # fp8 part, batch tile 0: u8c[p, k2, n] = shard[n, k2*P+p] * SU8
        u8c = ((shard[:NB, 0:K8] * SU8).reshape(NB, 2, P)
               .transpose(2, 1, 0).astype(f8np))
        head8c = np.ascontiguousarray(
            np.concatenate([u8c, W8c], axis=2))             # [128, 2, 1536]
        # fp16 pairs (blocks 2..7), batch tile 0
        ub0c = (shard[:NB, K8:].astype(np.float16)
                .reshape(NB, N_F16, P).transpose(1, 2, 0))  # [6, 128, 512]
        headc = np.ascontiguousarray(
            np.concatenate([ub0c, wt16], axis=2))           # [6, 128, 1536]
        # batch tiles 1-3
        ubrc = np.ascontiguousarray(
            shard[NB:, K8:].astype(np.float16)
                 .reshape(B_TILES - 1, NB, N_F16, P)
                 .transpose(0, 3, 2, 1)).reshape(B_TILES - 1, P,
                                                 N_F16 * NB)
        ubr8c = np.ascontiguousarray(
            (shard[NB:, 0:K8] * SU8).reshape(B_TILES - 1, NB, 2, P)
            .transpose(0, 3, 2, 1).astype(f8np))            # [3, 128, 2, 512]
        in_maps.append({"head8": head8c, "head": headc, "ubr": ubrc,
                        "ubr8": ubr8c, "bias": bias2})

    if "nc" not in _CACHE:
        _CACHE["nc"] = _build_nc()
    res = run_bass_kernel_spmd(_CACHE["nc"], in_maps, core_ids, trace=_trace)

    y = np.empty((BATCH, DOUT), np.float32)
    for c in core_ids:
        ybc = np.asarray(res.results[c]["yb"])
        y[c * B_SHARD:(c + 1) * B_SHARD] = (
            ybc.reshape(B_TILES, P, J_BLOCKS, NB).transpose(0, 3, 2, 1)
               .reshape(B_SHARD, DOUT).astype(np.float32))
    out = y.reshape(BATCH, 1, DOUT)
    if _trace:
        return out, res
    return out


# revision 54
# speedup vs baseline: 1.0005x; 1.0005x over previous
"""LRU (Linear Recurrent Unit) single-step forward on 8 Trainium2 NeuronCores.

Math: with seq-len 1 the whole LRU step collapses algebraically to one GEMM:
    y[b,:] = W @ u[b] + bias
(W, bias folded on host in float64; see _fold_params).  The batch GEMM runs
on the 8 NeuronCores, data-parallel over the batch: each core computes
y_shard^T = W @ u_shard^T (+bias), a 2048x1024x1024 GEMM.

Mixed-precision split (norm rel err 1.6e-2 vs the 2e-2 gate, verified
offline against the exact quantization):
  - Contraction range 0:256 runs in fp8e4m3 with MatmulPerfMode.DoubleRow
    (2 MACs/cell/cycle): operands scaled by 16 (u) and 16 (W).
  - Contraction range 256:1024 runs in fp16: u unscaled, W scaled by 256.
  - All partial products therefore carry exactly 2^8 (power-of-2 scales are
    exact in fp8/fp16), so both parts accumulate in ONE fp32 PSUM group;
    the PSUM->SBUF drain rescales by 2^-8 and adds the bias.
This cuts the PE stream from 256 to 32 DoubleRow + 192 fp16 matmuls
(~49us vs the 55.3us all-fp16 floor).

Pipeline (per core):
  - Combined (u-block, W-block) "pair" loads: one DMA per contraction
    block -> exactly one DMAHW-lane semaphore gates each K-outer matmul
    group (avoids Tile's wait-merging: 8 round-robin lanes, 1 wait slot
    per matmul).  The fp8 head pair is split so the first matmul group
    gates on 256 KiB.
  - Loads alternate across both HWDGE queues (sync ring first: measured
    first-byte ~0.65us vs ~1.5us for scalar's first use); u tiles for
    batch-tiles 1-3 ride per-deadline-chosen queues.
  - PE warm-up junk matmuls gated only by two tiny DVE memsets cover the
    DMA-latency launch window and release the HAM clock gate (~3.4us
    continuous-activity window) as the first data lands.
  - PSUM->SBUF drains on DVE (x2^-8, +bias, fp16 cast via tensor_scalar);
    stores per jb-pair alternate across both queues.  The very last
    matmul group is split into two half-width PSUM groups (ACT drains the
    first half in parallel; act-table preloaded by an early ACT drain),
    leaving ~380ns half-drain + one 32 KiB store after the final matmul.
y returns as fp16 and is upcast on host.

Remaining fixed overhead outside kernel control: the NEFF wrapper's
epilogue (~8.5us: one-at-a-time sweep zeroing all 256 semaphores) and the
~4-6us DMA-latency launch window (hidden behind warm-up).
"""

import numpy as np

BATCH, DIN, DSTATE, DOUT = 16384, 1024, 2048, 1024
N_CORES = 8
B_SHARD = BATCH // N_CORES  # 2048 rows per core
P = 128                     # SBUF partitions
NB = 512                    # batch tile (moving free dim, max 512 per PSUM bank)
I_BLOCKS = DIN // P         # 8 contraction blocks (block 0:2 fused in fp8)
J_BLOCKS = DOUT // P        # 8 output-row blocks
B_TILES = B_SHARD // NB     # 4 batch tiles per core
N_WARM = 17                 # PE warm-up matmuls (HAM clock-gate release)
F16_IB0 = 2                 # first fp16 contraction block index
N_F16 = I_BLOCKS - F16_IB0  # 6 fp16 blocks
SU8, SW8, SW16 = 16.0, 16.0, 256.0   # su8*sw8 == 1*sw16 == 2^8
SCALE_INV = 1.0 / 256.0

_CACHE = {}


def _build_nc():
    import concourse.mybir as mybir
    import concourse.tile as tile
    from concourse import bacc
    from concourse._compat import get_trn_type

    nc = bacc.Bacc(get_trn_type() or "TRN2", target_bir_lowering=False)
    f32 = mybir.dt.float32
    f16 = mybir.dt.float16
    f8 = mybir.dt.float8e4
    dr = mybir.MatmulPerfMode.DoubleRow

    # fp8 head: [p][k2][0:NB u8 | NB:NB+DOUT W8] for contraction 0:256
    head8 = nc.declare_dram_parameter("head8", [P, 2, NB + DOUT], f8,
                                      isOutput=False)
    # fp16 pairs for contraction blocks 2..7: [p][0:NB u | NB:NB+DOUT W]
    head = nc.declare_dram_parameter("head", [N_F16, P, NB + DOUT], f16,
                                     isOutput=False)
    # u batch-tiles 1-3: fp16 part (blocks 2..7) and fp8 part (0:256)
    ubr = nc.declare_dram_parameter("ubr", [B_TILES - 1, P, N_F16 * NB],
                                    f16, isOutput=False)
    ubr8 = nc.declare_dram_parameter("ubr8", [B_TILES - 1, P, 2, NB], f8,
                                     isOutput=False)
    bias = nc.declare_dram_parameter("bias", [P, J_BLOCKS], f32,
                                     isOutput=False)
    yb = nc.declare_dram_parameter("yb", [B_TILES, P, J_BLOCKS * NB], f16,
                                   isOutput=True)

    with tile.TileContext(nc) as tc:
        with (
            tc.tile_pool(name="consts", bufs=1) as consts,
            tc.tile_pool(name="upool", bufs=1) as upool,
            tc.tile_pool(name="ypool", bufs=1) as ypool,
            tc.tile_pool(name="psum", bufs=8, space="PSUM") as psum,
        ):
            warm_w = consts.tile([P, P], f16, tag="warm_w")
            warm_u = consts.tile([P, 256], f16, tag="warm_u")
            nc.vector.memset(warm_w[:], 0.0)
            nc.vector.memset(warm_u[:], 0.0)
            warm_p = psum.tile([P, NB], f32, tag="pt", name="warm_p")
            for _ in range(N_WARM):
                nc.tensor.matmul(warm_p[:, 0:256], warm_w[:], warm_u[:],
                                 start=True, stop=True)

            # fp8 head split: the first matmul group (jb0-3) gates on
            # u8 + W8 columns 0:512 (256 KiB)
            pair8_t = consts.tile([P, 2, NB + DOUT], f8, tag="pair8",
                                  name="pair8")
            h = NB + DOUT // 2
            nc.sync.dma_start(out=pair8_t[:, :, 0:h], in_=head8[:, :, 0:h])
            pair_tiles = []
            for hib in range(N_F16):
                pt_ = consts.tile([P, NB + DOUT], f16, tag=f"pair{hib}",
                                  name=f"pair{hib}")
                pair_tiles.append(pt_)
            nc.scalar.dma_start(out=pair_tiles[0][:], in_=head[0])
            nc.sync.dma_start(out=pair8_t[:, :, h:NB + DOUT],
                              in_=head8[:, :, h:NB + DOUT])
            nc.scalar.dma_start(out=pair_tiles[1][:], in_=head[1])
            nc.sync.dma_start(out=pair_tiles[2][:], in_=head[2])
            nc.scalar.dma_start(out=pair_tiles[3][:], in_=head[3])
            nc.sync.dma_start(out=pair_tiles[4][:], in_=head[4])
            nc.scalar.dma_start(out=pair_tiles[5][:], in_=head[5])
            bias_t = consts.tile([P, J_BLOCKS], f32, tag="bias")
            nc.sync.dma_start(out=bias_t[:], in_=bias[:])
            ur8_tiles = []
            ur_tiles = []
            for r in range(B_TILES - 1):
                u8t = upool.tile([P, 2, NB], f8, tag=f"ur8_{r}",
                                 name=f"ur8_{r}")
                ur8_tiles.append(u8t)
                urt = upool.tile([P, N_F16 * NB], f16, tag=f"ur{r}",
                                 name=f"ur{r}")
                ur_tiles.append(urt)
            # bt1 (tightest deadline) on sync; spread the rest
            nc.scalar.dma_start(out=ur8_tiles[0][:], in_=ubr8[0])
            nc.sync.dma_start(out=ur_tiles[0][:], in_=ubr[0])
            nc.scalar.dma_start(out=ur_tiles[1][:], in_=ubr[1])
            nc.sync.dma_start(out=ur8_tiles[1][:], in_=ubr8[1])
            nc.scalar.dma_start(out=ur8_tiles[2][:], in_=ubr8[2])
            nc.sync.dma_start(out=ur_tiles[2][:], in_=ubr[2])

            def w8_block(jb):
                return pair8_t[:, :, NB + jb * P:NB + (jb + 1) * P]

            def w_block(ib, jb):
                return pair_tiles[ib - F16_IB0][:, NB + jb * P:
                                                NB + (jb + 1) * P]

            def u0_block(ib):
                return pair_tiles[ib - F16_IB0][:, 0:NB]

            y_tiles = [ypool.tile([P, J_BLOCKS * NB], f16, tag=f"y{bt}",
                                  name=f"y{bt}")
                       for bt in range(B_TILES)]

            act_id = mybir.ActivationFunctionType.Identity
            op_mul = mybir.AluOpType.mult
            op_add = mybir.AluOpType.add

            def drain_store(bt, jb, pt):
                """PSUM -> SBUF: x2^-8, +bias, fp16 cast; then store."""
                yt = y_tiles[bt]
                if bt == 1 and jb == 1:
                    # early ACT drain preloads the activation table
                    nc.scalar.activation(yt[:, jb * NB:(jb + 1) * NB], pt[:],
                                         act_id, bias=bias_t[:, jb:jb + 1],
                                         scale=SCALE_INV)
                else:
                    nc.vector.tensor_scalar(yt[:, jb * NB:(jb + 1) * NB],
                                            pt[:], SCALE_INV,
                                            bias_t[:, jb:jb + 1],
                                            op_mul, op_add)
                if bt == B_TILES - 1:
                    q = nc.sync if jb % 2 == 0 else nc.scalar
                    q.dma_start(
                        out=yb[bt, :, jb * NB:(jb + 1) * NB],
                        in_=yt[:, jb * NB:(jb + 1) * NB])
                elif jb % 2 == 1:
                    nc.scalar.dma_start(
                        out=yb[bt, :, (jb - 1) * NB:(jb + 1) * NB],
                        in_=yt[:, (jb - 1) * NB:(jb + 1) * NB])

            # Batch tile 0 runs K-outer: the fp8 DoubleRow layer opens all 8
            # PSUM groups, then the 6 fp16 layers accumulate.
            pts = [psum.tile([P, NB], f32, tag="pt", name=f"pt_0_{jb}")
                   for jb in range(J_BLOCKS)]
            for jb in range(J_BLOCKS):
                nc.tensor.matmul(pts[jb][:], w8_block(jb),
                                 pair8_t[:, :, 0:NB],
                                 start=True, stop=False, perf_mode=dr)
            for ib in range(F16_IB0, I_BLOCKS):
                for jb in range(J_BLOCKS):
                    nc.tensor.matmul(
                        pts[jb][:],
                        w_block(ib, jb),
                        u0_block(ib),
                        start=False,
                        stop=(ib == I_BLOCKS - 1),
                    )
            for jb in range(J_BLOCKS):
                drain_store(0, jb, pts[jb])

            # Batch tiles 1-3 run in 4-group blocks: the DoubleRow matmuls of
            # a block issue back-to-back (isolated DR matmuls pace at ~407ns;
            # consecutive ones pipeline at ~250ns), then the fp16 layers,
            # then the block's drains (which overlap the next block).
            for bt in range(1, B_TILES):
                ur8 = ur8_tiles[bt - 1]
                ur = ur_tiles[bt - 1]
                last_bt = bt == B_TILES - 1
                blocks = ([(0, 1, 2, 3), (4, 5, 6)] if last_bt
                          else [(0, 1, 2, 3), (4, 5, 6, 7)])
                for blk in blocks:
                    bpts = {}
                    for jb in blk:
                        pt = psum.tile([P, NB], f32, tag="pt",
                                       name=f"pt_{bt}_{jb}")
                        nc.tensor.matmul(pt[:], w8_block(jb),
                                         ur8[:, :, 0:NB],
                                         start=True, stop=False,
                                         perf_mode=dr)
                        bpts[jb] = pt
                    for ib in range(F16_IB0, I_BLOCKS):
                        for jb in blk:
                            nc.tensor.matmul(
                                bpts[jb][:],
                                w_block(ib, jb),
                                ur[:, (ib - F16_IB0) * NB:
                                   (ib - F16_IB0 + 1) * NB],
                                start=False,
                                stop=(ib == I_BLOCKS - 1),
                            )
                    for jb in blk:
                        drain_store(bt, jb, bpts[jb])
                if last_bt:
                    # final jb7: two half-width PSUM groups (DRs back-to-back)
                    # for a short kernel tail after the last matmul
                    jb = J_BLOCKS - 1
                    hh = NB // 2
                    yt = y_tiles[bt]
                    pths = []
                    for half in range(2):
                        pth = psum.tile([P, NB], f32, tag="pt",
                                        name=f"pt_{bt}_{jb}_{half}")
                        nc.tensor.matmul(
                            pth[:, 0:hh], w8_block(jb),
                            ur8[:, :, half * hh:half * hh + hh],
                            start=True, stop=False, perf_mode=dr)
                        pths.append(pth)
                    for ib in range(F16_IB0, I_BLOCKS):
                        for half in range(2):
                            o = (ib - F16_IB0) * NB + half * hh
                            nc.tensor.matmul(
                                pths[half][:, 0:hh],
                                w_block(ib, jb),
                                ur[:, o:o + hh],
                                start=False,
                                stop=(ib == I_BLOCKS - 1),
                            )
                    for half in range(2):
                        c0 = jb * NB + half * hh
                        if half == 0:
                            nc.scalar.activation(
                                yt[:, c0:c0 + hh], pths[half][:, 0:hh],
                                act_id, bias=bias_t[:, jb:jb + 1],
                                scale=SCALE_INV)
                            nc.scalar.dma_start(
                                out=yb[bt, :, c0:c0 + hh],
                                in_=yt[:, c0:c0 + hh])
                        else:
                            nc.vector.tensor_scalar(
                                yt[:, c0:c0 + hh], pths[half][:, 0:hh],
                                SCALE_INV, bias_t[:, jb:jb + 1],
                                op_mul, op_add)
                            nc.sync.dma_start(
                                out=yb[bt, :, c0:c0 + hh],
                                in_=yt[:, c0:c0 + hh])
    nc.compile()
    return nc


def _fold_params(x_re, x_im, nu_log, theta_log, gamma_log, B_re, B_im, C_re, C_im, D):
    """Fold the LRU parameters into (W^T [DIN, DOUT], bias [DOUT]) in float64."""
    nu = np.asarray(nu_log, np.float64)
    th = np.exp(np.asarray(theta_log, np.float64))
    lam_mod = np.exp(-np.exp(nu))
    lam_re = lam_mod * np.cos(th)
    lam_im = lam_mod * np.sin(th)
    g = np.exp(np.asarray(gamma_log, np.float64))
    C_re64 = np.asarray(C_re, np.float64)
    C_im64 = np.asarray(C_im, np.float64)
    W = (2.0 * ((C_re64 * g) @ np.asarray(B_re, np.float64))
         - 2.0 * ((C_im64 * g) @ np.asarray(B_im, np.float64))
         + np.asarray(D, np.float64))  # [DOUT, DIN]
    xr = np.asarray(x_re, np.float64)
    xi = np.asarray(x_im, np.float64)
    lx_re = lam_re * xr - lam_im * xi
    lx_im = lam_re * xi + lam_im * xr
    bias = 2.0 * (C_re64 @ lx_re - C_im64 @ lx_im)  # [DOUT]
    return W.T.astype(np.float32).copy(), bias.astype(np.float32)


def kernel(u_in, x_re, x_im, nu_log, theta_log, gamma_log, B_re, B_im,
           C_re, C_im, D, _trace=False):
    from concourse.bass_utils import run_bass_kernel_spmd
    import concourse.mybir as mybir

    f8np = mybir.dt.np(mybir.dt.float8e4)

    wt_host, bias_host = _fold_params(
        x_re, x_im, nu_log, theta_log, gamma_log, B_re, B_im, C_re, C_im, D)
    bias2 = np.ascontiguousarray(bias_host.reshape(J_BLOCKS, P).T)  # [128, 8]

    K8 = F16_IB0 * P   # 256 contraction indices in fp8
    # W8c[p, k2, j] = W^T[k2*P+p, j] * SW8   (fp8)
    W8c = ((wt_host[0:K8] * SW8).reshape(2, P, DOUT)
           .transpose(1, 0, 2).astype(f8np))
    # wt16[hib, p, j] = W^T[K8+hib*P+p, j] * SW16  (fp16)
    wt16 = ((wt_host[K8:] * SW16).astype(np.float16)
            .reshape(N_F16, P, DOUT))

    u32 = np.asarray(u_in, np.float32).reshape(BATCH, DIN)
    core_ids = list(range(N_CORES))
    in_maps = []
    for c in core_ids:
        shard = u32[c * B_SHARD:(c + 1) * B_SHARD]          # [2048, 1024]
        # fp8 part, batch tile 0: u8c[p, k2, n] = shard[n, k2*P+p] * SU8
        u8c = ((shard[:NB, 0:K8] * SU8).reshape(NB, 2, P)
               .transpose(2, 1, 0).astype(f8np))
        head8c = np.ascontiguousarray(
            np.concatenate([u8c, W8c], axis=2))             # [128, 2, 1536]
        # fp16 pairs (blocks 2..7), batch tile 0
        ub0c = (shard[:NB, K8:].astype(np.float16)
                .reshape(NB, N_F16, P).transpose(1, 2, 0))  # [6, 128, 512]
        headc = np.ascontiguousarray(
            np.concatenate([ub0c, wt16], axis=2))           # [6, 128, 1536]
        # batch tiles 1-3
        ubrc = np.ascontiguousarray(
            shard[NB:, K8:].astype(np.float16)
                 .reshape(B_TILES - 1, NB, N_F16, P)
                 .transpose(0, 3, 2, 1)).reshape(B_TILES - 1, P,
                                                 N_F16 * NB)
        ubr8c = np.ascontiguousarray(
            (shard[NB:, 0:K8] * SU8).reshape(B_TILES - 1, NB, 2, P)
            .transpose(0, 3, 2, 1).astype(f8np))            # [3, 128, 2, 512]
        in_maps.append({"head8": head8c, "head": headc, "ubr": ubrc,
                        "ubr8": ubr8c, "bias": bias2})

    if "nc" not in _CACHE:
        _CACHE["nc"] = _build_nc()
    res = run_bass_kernel_spmd(_CACHE["nc"], in_maps, core_ids, trace=_trace)

    y = np.empty((BATCH, DOUT), np.float32)
    for c in core_ids:
        ybc = np.asarray(res.results[c]["yb"])
        y[c * B_SHARD:(c + 1) * B_SHARD] = (
            ybc.reshape(B_TILES, P, J_BLOCKS, NB).transpose(0, 3, 2, 1)
               .reshape(B_SHARD, DOUT).astype(np.float32))
    out = y.reshape(BATCH, 1, DOUT)
    if _trace:
        return out, res
    return out


# revision 55
# speedup vs baseline: 1.0055x; 1.0050x over previous
"""LRU (Linear Recurrent Unit) single-step forward on 8 Trainium2 NeuronCores.

Math: with seq-len 1 the whole LRU step collapses algebraically to one GEMM:
    y[b,:] = W @ u[b] + bias
(W, bias folded on host in float64; see _fold_params).  The batch GEMM runs
on the 8 NeuronCores, data-parallel over the batch: each core computes
y_shard^T = W @ u_shard^T (+bias), a 2048x1024x1024 GEMM.

Mixed-precision split (norm rel err 1.6e-2 vs the 2e-2 gate, verified
offline against the exact quantization):
  - Contraction range 0:256 runs in fp8e4m3 with MatmulPerfMode.DoubleRow
    (2 MACs/cell/cycle): operands scaled by 16 (u) and 16 (W).
  - Contraction range 256:1024 runs in fp16: u unscaled, W scaled by 256.
  - All partial products therefore carry exactly 2^8 (power-of-2 scales are
    exact in fp8/fp16), so both parts accumulate in ONE fp32 PSUM group;
    the PSUM->SBUF drain rescales by 2^-8 and adds the bias.
This cuts the PE stream from 256 to 32 DoubleRow + 192 fp16 matmuls
(~49us vs the 55.3us all-fp16 floor).

Pipeline (per core):
  - Combined (u-block, W-block) "pair" loads: one DMA per contraction
    block -> exactly one DMAHW-lane semaphore gates each K-outer matmul
    group (avoids Tile's wait-merging: 8 round-robin lanes, 1 wait slot
    per matmul).  The fp8 head pair is split so the first matmul group
    gates on 256 KiB.
  - Loads alternate across both HWDGE queues (sync ring first: measured
    first-byte ~0.65us vs ~1.5us for scalar's first use); u tiles for
    batch-tiles 1-3 ride per-deadline-chosen queues.
  - PE warm-up junk matmuls gated only by two tiny DVE memsets cover the
    DMA-latency launch window and release the HAM clock gate (~3.4us
    continuous-activity window) as the first data lands.
  - PSUM->SBUF drains on DVE (x2^-8, +bias, fp16 cast via tensor_scalar);
    stores per jb-pair alternate across both queues.  The very last
    matmul group is split into two half-width PSUM groups (ACT drains the
    first half in parallel; act-table preloaded by an early ACT drain),
    leaving ~380ns half-drain + one 32 KiB store after the final matmul.
y returns as fp16 and is upcast on host.

Remaining fixed overhead outside kernel control: the NEFF wrapper's
epilogue (~8.5us: one-at-a-time sweep zeroing all 256 semaphores) and the
~4-6us DMA-latency launch window (hidden behind warm-up).
"""

import numpy as np

BATCH, DIN, DSTATE, DOUT = 16384, 1024, 2048, 1024
N_CORES = 8
B_SHARD = BATCH // N_CORES  # 2048 rows per core
P = 128                     # SBUF partitions
NB = 512                    # batch tile (moving free dim, max 512 per PSUM bank)
I_BLOCKS = DIN // P         # 8 contraction blocks (block 0:2 fused in fp8)
J_BLOCKS = DOUT // P        # 8 output-row blocks
B_TILES = B_SHARD // NB     # 4 batch tiles per core
N_WARM = 16                 # PE warm-up matmuls (HAM clock-gate release)
F16_IB0 = 2                 # first fp16 contraction block index
N_F16 = I_BLOCKS - F16_IB0  # 6 fp16 blocks
SU8, SW8, SW16 = 16.0, 16.0, 256.0   # su8*sw8 == 1*sw16 == 2^8
SCALE_INV = 1.0 / 256.0

_CACHE = {}


def _build_nc():
    import concourse.mybir as mybir
    import concourse.tile as tile
    from concourse import bacc
    from concourse._compat import get_trn_type

    nc = bacc.Bacc(get_trn_type() or "TRN2", target_bir_lowering=False)
    f32 = mybir.dt.float32
    f16 = mybir.dt.float16
    f8 = mybir.dt.float8e4
    dr = mybir.MatmulPerfMode.DoubleRow

    # fp8 head: [p][k2][0:NB u8 | NB:NB+DOUT W8] for contraction 0:256
    head8 = nc.declare_dram_parameter("head8", [P, 2, NB + DOUT], f8,
                                      isOutput=False)
    # fp16 pairs for contraction blocks 2..7: [p][0:NB u | NB:NB+DOUT W]
    head = nc.declare_dram_parameter("head", [N_F16, P, NB + DOUT], f16,
                                     isOutput=False)
    # u batch-tiles 1-3: fp16 part (blocks 2..7) and fp8 part (0:256)
    ubr = nc.declare_dram_parameter("ubr", [B_TILES - 1, P, N_F16 * NB],
                                    f16, isOutput=False)
    ubr8 = nc.declare_dram_parameter("ubr8", [B_TILES - 1, P, 2, NB], f8,
                                     isOutput=False)
    bias = nc.declare_dram_parameter("bias", [P, J_BLOCKS], f32,
                                     isOutput=False)
    yb = nc.declare_dram_parameter("yb", [B_TILES, P, J_BLOCKS * NB], f16,
                                   isOutput=True)

    with tile.TileContext(nc) as tc:
        with (
            tc.tile_pool(name="consts", bufs=1) as consts,
            tc.tile_pool(name="upool", bufs=1) as upool,
            tc.tile_pool(name="ypool", bufs=1) as ypool,
            tc.tile_pool(name="psum", bufs=8, space="PSUM") as psum,
        ):
            warm_w = consts.tile([P, P], f16, tag="warm_w")
            warm_u = consts.tile([P, 256], f16, tag="warm_u")
            nc.vector.memset(warm_w[:], 0.0)
            nc.vector.memset(warm_u[:], 0.0)
            warm_p = psum.tile([P, NB], f32, tag="pt", name="warm_p")
            for _ in range(N_WARM):
                nc.tensor.matmul(warm_p[:, 0:256], warm_w[:], warm_u[:],
                                 start=True, stop=True)

            # fp8 head split: the first matmul group (jb0-3) gates on
            # u8 + W8 columns 0:512 (256 KiB)
            pair8_t = consts.tile([P, 2, NB + DOUT], f8, tag="pair8",
                                  name="pair8")
            h = NB + DOUT // 2
            nc.sync.dma_start(out=pair8_t[:, :, 0:h], in_=head8[:, :, 0:h])
            pair_tiles = []
            for hib in range(N_F16):
                pt_ = consts.tile([P, NB + DOUT], f16, tag=f"pair{hib}",
                                  name=f"pair{hib}")
                pair_tiles.append(pt_)
            nc.scalar.dma_start(out=pair_tiles[0][:], in_=head[0])
            nc.sync.dma_start(out=pair8_t[:, :, h:NB + DOUT],
                              in_=head8[:, :, h:NB + DOUT])
            nc.scalar.dma_start(out=pair_tiles[1][:], in_=head[1])
            nc.sync.dma_start(out=pair_tiles[2][:], in_=head[2])
            nc.scalar.dma_start(out=pair_tiles[3][:], in_=head[3])
            nc.sync.dma_start(out=pair_tiles[4][:], in_=head[4])
            nc.scalar.dma_start(out=pair_tiles[5][:], in_=head[5])
            bias_t = consts.tile([P, J_BLOCKS], f32, tag="bias")
            nc.sync.dma_start(out=bias_t[:], in_=bias[:])
            ur8_tiles = []
            ur_tiles = []
            for r in range(B_TILES - 1):
                u8t = upool.tile([P, 2, NB], f8, tag=f"ur8_{r}",
                                 name=f"ur8_{r}")
                ur8_tiles.append(u8t)
                urt = upool.tile([P, N_F16 * NB], f16, tag=f"ur{r}",
                                 name=f"ur{r}")
                ur_tiles.append(urt)
            # bt1 (tightest deadline) on sync; spread the rest
            nc.scalar.dma_start(out=ur8_tiles[0][:], in_=ubr8[0])
            nc.sync.dma_start(out=ur_tiles[0][:], in_=ubr[0])
            nc.scalar.dma_start(out=ur_tiles[1][:], in_=ubr[1])
            nc.sync.dma_start(out=ur8_tiles[1][:], in_=ubr8[1])
            nc.scalar.dma_start(out=ur8_tiles[2][:], in_=ubr8[2])
            nc.sync.dma_start(out=ur_tiles[2][:], in_=ubr[2])

            def w8_block(jb):
                return pair8_t[:, :, NB + jb * P:NB + (jb + 1) * P]

            def w_block(ib, jb):
                return pair_tiles[ib - F16_IB0][:, NB + jb * P:
                                                NB + (jb + 1) * P]

            def u0_block(ib):
                return pair_tiles[ib - F16_IB0][:, 0:NB]

            y_tiles = [ypool.tile([P, J_BLOCKS * NB], f16, tag=f"y{bt}",
                                  name=f"y{bt}")
                       for bt in range(B_TILES)]

            act_id = mybir.ActivationFunctionType.Identity
            op_mul = mybir.AluOpType.mult
            op_add = mybir.AluOpType.add

            def drain_store(bt, jb, pt):
                """PSUM -> SBUF: x2^-8, +bias, fp16 cast; then store."""
                yt = y_tiles[bt]
                if bt == 1 and jb == 1:
                    # early ACT drain preloads the activation table
                    nc.scalar.activation(yt[:, jb * NB:(jb + 1) * NB], pt[:],
                                         act_id, bias=bias_t[:, jb:jb + 1],
                                         scale=SCALE_INV)
                else:
                    nc.vector.tensor_scalar(yt[:, jb * NB:(jb + 1) * NB],
                                            pt[:], SCALE_INV,
                                            bias_t[:, jb:jb + 1],
                                            op_mul, op_add)
                if bt == B_TILES - 1:
                    q = nc.sync if jb % 2 == 0 else nc.scalar
                    q.dma_start(
                        out=yb[bt, :, jb * NB:(jb + 1) * NB],
                        in_=yt[:, jb * NB:(jb + 1) * NB])
                elif jb % 2 == 1:
                    nc.scalar.dma_start(
                        out=yb[bt, :, (jb - 1) * NB:(jb + 1) * NB],
                        in_=yt[:, (jb - 1) * NB:(jb + 1) * NB])

            # Batch tile 0 runs K-outer: the fp8 DoubleRow layer opens all 8
            # PSUM groups, then the 6 fp16 layers accumulate.
            pts = [psum.tile([P, NB], f32, tag="pt", name=f"pt_0_{jb}")
                   for jb in range(J_BLOCKS)]
            for jb in range(J_BLOCKS):
                nc.tensor.matmul(pts[jb][:], w8_block(jb),
                                 pair8_t[:, :, 0:NB],
                                 start=True, stop=False, perf_mode=dr)
            for ib in range(F16_IB0, I_BLOCKS):
                for jb in range(J_BLOCKS):
                    nc.tensor.matmul(
                        pts[jb][:],
                        w_block(ib, jb),
                        u0_block(ib),
                        start=False,
                        stop=(ib == I_BLOCKS - 1),
                    )
            for jb in range(J_BLOCKS):
                drain_store(0, jb, pts[jb])

            # Batch tiles 1-3 run in 4-group blocks: the DoubleRow matmuls of
            # a block issue back-to-back (isolated DR matmuls pace at ~407ns;
            # consecutive ones pipeline at ~250ns), then the fp16 layers,
            # then the block's drains (which overlap the next block).
            for bt in range(1, B_TILES):
                ur8 = ur8_tiles[bt - 1]
                ur = ur_tiles[bt - 1]
                last_bt = bt == B_TILES - 1
                blocks = ([(0, 1, 2, 3), (4, 5, 6)] if last_bt
                          else [(0, 1, 2, 3), (4, 5, 6, 7)])
                for blk in blocks:
                    bpts = {}
                    for jb in blk:
                        pt = psum.tile([P, NB], f32, tag="pt",
                                       name=f"pt_{bt}_{jb}")
                        nc.tensor.matmul(pt[:], w8_block(jb),
                                         ur8[:, :, 0:NB],
                                         start=True, stop=False,
                                         perf_mode=dr)
                        bpts[jb] = pt
                    for ib in range(F16_IB0, I_BLOCKS):
                        for jb in blk:
                            nc.tensor.matmul(
                                bpts[jb][:],
                                w_block(ib, jb),
                                ur[:, (ib - F16_IB0) * NB:
                                   (ib - F16_IB0 + 1) * NB],
                                start=False,
                                stop=(ib == I_BLOCKS - 1),
                            )
                    for jb in blk:
                        drain_store(bt, jb, bpts[jb])
                if last_bt:
                    # final jb7: two half-width PSUM groups (DRs back-to-back)
                    # for a short kernel tail after the last matmul
                    jb = J_BLOCKS - 1
                    hh = NB // 2
                    yt = y_tiles[bt]
                    pths = []
                    for half in range(2):
                        pth = psum.tile([P, NB], f32, tag="pt",
                                        name=f"pt_{bt}_{jb}_{half}")
                        nc.tensor.matmul(
                            pth[:, 0:hh], w8_block(jb),
                            ur8[:, :, half * hh:half * hh + hh],
                            start=True, stop=False, perf_mode=dr)
                        pths.append(pth)
                    for ib in range(F16_IB0, I_BLOCKS):
                        for half in range(2):
                            o = (ib - F16_IB0) * NB + half * hh
                            nc.tensor.matmul(
                                pths[half][:, 0:hh],
                                w_block(ib, jb),
                                ur[:, o:o + hh],
                                start=False,
                                stop=(ib == I_BLOCKS - 1),
                            )
                    for half in range(2):
                        c0 = jb * NB + half * hh
                        if half == 0:
                            nc.scalar.activation(
                                yt[:, c0:c0 + hh], pths[half][:, 0:hh],
                                act_id, bias=bias_t[:, jb:jb + 1],
                                scale=SCALE_INV)
                            nc.scalar.dma_start(
                                out=yb[bt, :, c0:c0 + hh],
                                in_=yt[:, c0:c0 + hh])
                        else:
                            nc.vector.tensor_scalar(
                                yt[:, c0:c0 + hh], pths[half][:, 0:hh],
                                SCALE_INV, bias_t[:, jb:jb + 1],
                                op_mul, op_add)
                            nc.sync.dma_start(
                                out=yb[bt, :, c0:c0 + hh],
                                in_=yt[:, c0:c0 + hh])
    nc.compile()
    return nc


def _fold_params(x_re, x_im, nu_log, theta_log, gamma_log, B_re, B_im, C_re, C_im, D):
    """Fold the LRU parameters into (W^T [DIN, DOUT], bias [DOUT]) in float64."""
    nu = np.asarray(nu_log, np.float64)
    th = np.exp(np.asarray(theta_log, np.float64))
    lam_mod = np.exp(-np.exp(nu))
    lam_re = lam_mod * np.cos(th)
    lam_im = lam_mod * np.sin(th)
    g = np.exp(np.asarray(gamma_log, np.float64))
    C_re64 = np.asarray(C_re, np.float64)
    C_im64 = np.asarray(C_im, np.float64)
    W = (2.0 * ((C_re64 * g) @ np.asarray(B_re, np.float64))
         - 2.0 * ((C_im64 * g) @ np.asarray(B_im, np.float64))
         + np.asarray(D, np.float64))  # [DOUT, DIN]
    xr = np.asarray(x_re, np.float64)
    xi = np.asarray(x_im, np.float64)
    lx_re = lam_re * xr - lam_im * xi
    lx_im = lam_re * xi + lam_im * xr
    bias = 2.0 * (C_re64 @ lx_re - C_im64 @ lx_im)  # [DOUT]
    return W.T.astype(np.float32).copy(), bias.astype(np.float32)


def kernel(u_in, x_re, x_im, nu_log, theta_log, gamma_log, B_re, B_im,
           C_re, C_im, D, _trace=False):
    from concourse.bass_utils import run_bass_kernel_spmd
    import concourse.mybir as mybir

    f8np = mybir.dt.np(mybir.dt.float8e4)

    wt_host, bias_host = _fold_params(
        x_re, x_im, nu_log, theta_log, gamma_log, B_re, B_im, C_re, C_im, D)
    bias2 = np.ascontiguousarray(bias_host.reshape(J_BLOCKS, P).T)  # [128, 8]

    K8 = F16_IB0 * P   # 256 contraction indices in fp8
    # W8c[p, k2, j] = W^T[k2*P+p, j] * SW8   (fp8)
    W8c = ((wt_host[0:K8] * SW8).reshape(2, P, DOUT)
           .transpose(1, 0, 2).astype(f8np))
    # wt16[hib, p, j] = W^T[K8+hib*P+p, j] * SW16  (fp16)
    wt16 = ((wt_host[K8:] * SW16).astype(np.float16)
            .reshape(N_F16, P, DOUT))

    u32 = np.asarray(u_in, np.float32).reshape(BATCH, DIN)
    core_ids = list(range(N_CORES))
    in_maps = []
    for c in core_ids:
        shard = u32[c * B_SHARD:(c + 1) * B_SHARD]          # [2048, 1024]
        # fp8 part, batch tile 0: u8c[p, k2, n] = shard[n, k2*P+p] * SU8
        u8c = ((shard[:NB, 0:K8] * SU8).reshape(NB, 2, P)
               .transpose(2, 1, 0).astype(f8np))
        head8c = np.ascontiguousarray(
            np.concatenate([u8c, W8c], axis=2))             # [128, 2, 1536]
        # fp16 pairs (blocks 2..7), batch tile 0
        ub0c = (shard[:NB, K8:].astype(np.float16)
                .reshape(NB, N_F16, P).transpose(1, 2, 0))  # [6, 128, 512]
        headc = np.ascontiguousarray(
            np.concatenate([ub0c, wt16], axis=2))           # [6, 128, 1536]
        # batch tiles 1-3
        ubrc = np.ascontiguousarray(
            shard[NB:, K8:].astype(np.float16)
                 .reshape(B_TILES - 1, NB, N_F16, P)
                 .transpose(0, 3, 2, 1)).reshape(B_TILES - 1, P,
                                                 N_F16 * NB)
        ubr8c = np.ascontiguousarray(
            (shard[NB:, 0:K8] * SU8).reshape(B_TILES - 1, NB, 2, P)
            .transpose(0, 3, 2, 1).astype(f8np))            # [3, 128, 2, 512]
        in_maps.append({"head8": head8c, "head": headc, "ubr": ubrc,
                        "ubr8": ubr8c, "bias": bias2})

    if "nc" not in _CACHE:
        _CACHE["nc"] = _build_nc()
    res = run_bass_kernel_spmd(_CACHE["nc"], in_maps, core_ids, trace=_trace)

    y = np.empty((BATCH, DOUT), np.float32)
    for c in core_ids:
        ybc = np.asarray(res.results[c]["yb"])
        y[c * B_SHARD:(c + 1) * B_SHARD] = (
            ybc.reshape(B_TILES, P, J_BLOCKS, NB).transpose(0, 3, 2, 1)
               .reshape(B_SHARD, DOUT).astype(np.float32))
    out = y.reshape(BATCH, 1, DOUT)
    if _trace:
        return out, res
    return out
